# revision 55
# baseline (speedup 1.0000x reference)
"""GravNet layer Bass kernel for Trainium2, 8 NeuronCores (data-parallel over batch).

Wall time through the axon tunnel is dominated by tunnel round-trip latency
(~75-110ms per blocking sync) plus per-MB transfer cost (~10-20ms/MB), not
device compute (<1ms). This version is built around one round trip per call
and minimum bytes each way:

  * The jax.jit(shard_map(bass_exec)) wrapper is built ONCE and cached --
    run_bass_kernel_spmd re-creates it every call, paying a re-trace +
    persistent-cache lookup (~25ms) per call.
  * ONE merged input tensor per core (uint8-quantized feats + fp16 coords
    pack bit-cast into byte rows) -> a single device_put; ONE merged output
    tensor (int8-quantized wmean + its fp16 scales bit-cast into extra
    rows) -> 8 fetches instead of 16+ per call.
  * Donated output buffers are created ON DEVICE by a cached jitted zeros fn
    pre-dispatched at the END of the previous call -- no H2D upload, no
    dispatch latency on the current call.
  * Output shards are fetched with copy_to_host_async issued immediately
    after dispatch: wait-for-ready and D2H collapse into a single round trip
    (block_until_ready + asarray would be two).
  * Quantization: feats ride as uint8 u = round(f/s)+128 with a per-node
    fp16 scale s = absmax/126.5 (the +128.5 host trick makes uint8
    truncation equal round-to-nearest); wmean returns as per-node-scaled
    int8 the same way. Both quantizations together cost 2.3e-3 final rel
    err vs the 2e-2 gate (the host MLP input `feats` stays exact f32; only
    the neighbor-aggregation payload is quantized).
  * The host half of the MLP that doesn't need device results
    (feats @ W1[:64] + b1) runs during the tunnel wait, and the per-batch
    MLP tail (wmean @ W1[64:], relu, @W2) is pipelined against the
    per-shard download stream.

Host (~0.1% of FLOPs): coords = x@W_space, feats = x@W_feat, quantization,
and the final 2-layer MLP in f32 (more accurate than a device fp16 MLP and
cheaper than downloading a 128-wide result). The d2 expansion uses an fp16
hi/lo split of coords and |c|^2 over a 16-row contraction (2 hi*hi + 2 hi*lo
+ 2 lo*hi - n2 terms), so the PE's exact fp16 products + f32 PSUM
accumulation reproduce s = -d2 to ~1e-6 -- fp32 PE matmul (fp32r) and plain
fp16 coords both lose enough precision to flip kNN selections vs the
reference (~1e-2 rel err). Coords are pre-scaled by CSCALE so the hi/lo
residuals stay out of fp16-subnormal range (slow numpy conversions).

Device (per core, one batch element):
  Staging: dequantize feats (u-128)*s -> fp16 [feats|1]; build A/B
      expansion rows from the 10 uploaded coord rows.
  Loop1 (t in 16): s row-tile via matmul, w = exp(10/CSCALE^2 * s) in f32;
      top-8 twice (max8 + match_replace + max8) then an exact same-side
      compare w >= m2[:,7] keeps exactly the row-wise top-16 (f32, no
      ties); masked weights stored fp16.
  Loop2 (t in 16): PE-transpose the 16 blocks of the masked row-tile (exact
      for fp16 values) -> lhsT; aggregate against [feats|1] with PSUM
      accumulation; weighted mean -> per-node int8 quant + fp16 scale.
Output tile t needs exactly the transposed blocks of masked row-tile t, so
there is no index gather anywhere. Biases b1/b2 are applied on the host;
mask zeroes feats on the host (all-ones in this problem's spec).
"""

# Persistent XLA compilation cache so a cold process only pays neuronx-cc
# once per executable across runs. jax may already be initialized by the
# site hook, so set via config.update, not env vars.
import jax

jax.config.update("jax_compilation_cache_dir", "/tmp/jax_comp_cache")
jax.config.update("jax_persistent_cache_min_compile_time_secs", 0.0)
jax.config.update("jax_persistent_cache_min_entry_size_bytes", 0)

import numpy as np
import jax.numpy as jnp
from jax.sharding import Mesh, NamedSharding, PartitionSpec

import concourse.bass as bass
import concourse.bacc as bacc_mod
import concourse.bass2jax as bass2jax
import concourse.mybir as mybir
import concourse.tile as tile
from concourse.bass import ds
from concourse.masks import make_identity

# the deprecated experimental entry point still accepts check_rep (it's what
# bass2jax itself uses); jax.shard_map renamed it to check_vma
from jax.experimental.shard_map import shard_map

P = 128
N = 2048
DIN = 128
DS = 4
DP = 64
DOUT = 128
NT = N // P          # 16 row tiles
FREE = 512
JC = N // FREE       # 4 psum-bank chunks
B = 8
dt = mybir.dt
AF = mybir.ActivationFunctionType
ALU = mybir.AluOpType
F16 = dt.float16
F32 = dt.float32

# Coords are scaled by CSCALE (a power of two) before the fp16 hi/lo split:
# kNN ordering is scale-invariant and the device folds 1/CSCALE^2 into the
# exp() scale, but the split residuals move from ~2e-5 (fp16 SUBNORMAL --
# numpy's slow conversion path, ~3.5ms per call) to ~1.3e-3 (normal, fast).
# 64 keeps n2*CSCALE^2 (max ~6.5e3) well inside fp16 range.
CSCALE = 64.0

# packed fp16 coords-input rows (width 64)
R_HI = 0                   # [128, 64]   coords hi  [4, 2048]
R_LO = R_HI + 128          # [128, 64]   coords lo  [4, 2048]
R_N2 = R_LO + 128          # [64, 64]    [-n2_hi; -n2_lo] [2, 2048]
R_FS = R_N2 + 64           # [32, 64]    feat scales fp16 [P, NT] row-major
RC_END = R_FS + 32         # 352


def build_gravnet(nc: bass.Bass, debug: bool = False):
    # single merged input: N rows of uint8 feats + 2*RC_END rows carrying the
    # fp16 coords pack bit-cast to byte pairs (one device_put per call)
    pk_d = nc.dram_tensor("pk", [N + 2 * RC_END, DP], dt.uint8, kind="ExternalInput")
    pkf_d = pk_d[0:N, :]
    pk16_d = pk_d.bitcast(F16)          # [N + 2*RC_END, 32] fp16 view

    def pkc_view(row, n_rows):
        """fp16 view [2*n_rows, 32] of pkc-layout rows [row, row+n_rows)."""
        return pk16_d[N + 2 * row:N + 2 * (row + n_rows), :]
    # single output: N rows of int8 wmean + 64 rows carrying the per-node
    # fp16 scales bit-cast to int8 pairs (merging them into one tensor saves
    # 8 extra per-device fetch RPCs through the tunnel)
    out8_d = nc.dram_tensor("out8", [N + 64, DP], dt.int8, kind="ExternalOutput")
    if debug:
        dbg_w = nc.dram_tensor("dbg_w", [P, N], F16, kind="ExternalOutput")
        dbg_lhs = nc.dram_tensor("dbg_lhs", [P, NT * P], F16, kind="ExternalOutput")

    with tile.TileContext(nc) as tc:
        with (
            tc.tile_pool(name="big", bufs=1) as big,
            tc.tile_pool(name="ps_s", bufs=1, space="PSUM") as ps_s,
            tc.tile_pool(name="ps_agg", bufs=1, space="PSUM") as ps_agg,
            tc.tile_pool(name="ps_mlp", bufs=1, space="PSUM") as ps_mlp,
        ):
            # ---- constants / staged inputs ----
            identh = big.tile([P, P], F16)
            make_identity(nc, identh)

            f8_sb = big.tile([P, NT, DP], dt.uint8)
            nc.sync.dma_start(
                f8_sb, pkf_d[:, :].rearrange("(t p) d -> p t d", p=P)
            )
            # feat scales ride in pkc: [P, NT] fp16 row-major = [32, 64] rows;
            # element (p, t) sits at dram (p//4, 16*(p%4)+t)
            fs16_sb = big.tile([P, NT], F16)
            nc.sync.dma_start(
                fs16_sb,
                pkc_view(R_FS, 32).rearrange(
                    "(r pb1) (pb0 t) -> (r pb1 pb0) t", pb1=2, pb0=2
                ),
            )
            fs_sb = big.tile([P, NT], F32)
            nc.vector.tensor_copy(fs_sb, fs16_sb)
            feats_sb = big.tile([P, NT, DP + 1], F16)
            nc.vector.memset(feats_sb[:, :, DP:DP + 1], 1.0)
            # dequant: feats = (u - 128) * s_node, one fused op per tile
            for t in range(NT):
                nc.vector.tensor_scalar(
                    feats_sb[:, t, 0:DP],
                    f8_sb[:, t, :],
                    -128.0,
                    scalar2=fs_sb[:, t:t + 1],
                    op0=ALU.add,
                    op1=ALU.mult,
                )

            # A/B expansion rows rebuilt on device from the minimal 10 rows:
            # A = [hi hi lo -n2hi -n2lo 1 1], B = [2hi 2lo 2hi 1 1 -n2hi -n2lo]
            hi_t = big.tile([4, N], F16)
            nc.sync.dma_start(
                hi_t,
                pkc_view(R_HI, 128).rearrange(
                    "(r jh jl1) jl0 -> r (jh jl1 jl0)", r=4, jh=32, jl1=2
                ),
            )
            lo_t = big.tile([4, N], F16)
            nc.sync.dma_start(
                lo_t,
                pkc_view(R_LO, 128).rearrange(
                    "(r jh jl1) jl0 -> r (jh jl1 jl0)", r=4, jh=32, jl1=2
                ),
            )
            n2n_t = big.tile([2, N], F16)
            nc.sync.dma_start(
                n2n_t,
                pkc_view(R_N2, 64).rearrange(
                    "(r jh jl1) jl0 -> r (jh jl1 jl0)", r=2, jh=32, jl1=2
                ),
            )
            hi2_t = big.tile([4, N], F16)
            nc.scalar.activation(hi2_t, hi_t, AF.Copy, scale=2.0)  # exact in fp16
            lo2_t = big.tile([4, N], F16)
            nc.scalar.activation(lo2_t, lo_t, AF.Copy, scale=2.0)
            ones2 = big.tile([2, N], F16)
            nc.vector.memset(ones2, 1.0)
            A_sb = big.tile([16, N], F16)
            B_sb = big.tile([16, N], F16)
            nc.sync.dma_start(A_sb[0:4, :], hi_t)
            nc.sync.dma_start(A_sb[4:8, :], hi_t)
            nc.sync.dma_start(A_sb[8:12, :], lo_t)
            nc.sync.dma_start(A_sb[12:14, :], n2n_t)
            nc.sync.dma_start(A_sb[14:16, :], ones2)
            nc.sync.dma_start(B_sb[0:4, :], hi2_t)
            nc.sync.dma_start(B_sb[4:8, :], lo2_t)
            nc.sync.dma_start(B_sb[8:12, :], hi2_t)
            nc.sync.dma_start(B_sb[12:14, :], ones2)
            nc.sync.dma_start(B_sb[14:16, :], n2n_t)

            # ---- persistent state ----
            wm_all = big.tile([P, NT * N], F16)    # masked weight row-tiles
            o8_all = big.tile([P, NT * DP], dt.int8)   # quantized wmean tiles
            s_all = big.tile([P, NT], F16)             # per-node wmean scales

            # scratch (fixed addresses; loop back-edge serializes iterations)
            a_t = big.tile([16, P], F16)
            w_t = big.tile([P, N], F32)
            m1 = big.tile([P, 8], F32)
            m2 = big.tile([P, 8], F32)
            w1z = big.tile([P, N], F32)
            sel = big.tile([P, N], F32)
            wmt_t = big.tile([P, N], F16)
            lhs_cols = big.tile([P, NT, P], F16)   # transposed masked weights
            recip = big.tile([P, 1], F32)
            wmf = big.tile([P, DP], F32)
            amax = big.tile([P, 1], F32)
            s32 = big.tile([P, 1], F32)
            rq = big.tile([P, 1], F32)

            s_ps = ps_s.tile([P, N], F32)                  # 4 banks
            agg_ps = ps_agg.tile([P, DP + 1], F32)         # 1 bank

            # ---- Loop1: distances, exp, exact row-side top-16 mask ----
            with tc.For_i(0, NT, 1) as t:
                nc.sync.dma_start(a_t, A_sb[:, ds(t * P, P)])
                for c in range(JC):
                    nc.tensor.matmul(
                        s_ps[:, c * FREE:(c + 1) * FREE],
                        lhsT=a_t,
                        rhs=B_sb[:, c * FREE:(c + 1) * FREE],
                        start=True, stop=True,
                    )
                for c in range(JC):
                    nc.scalar.activation(
                        w_t[:, c * FREE:(c + 1) * FREE],
                        s_ps[:, c * FREE:(c + 1) * FREE],
                        AF.Exp, scale=10.0 / (CSCALE * CSCALE),
                    )
                nc.vector.max(m1, w_t)
                nc.vector.match_replace(
                    w1z, in_to_replace=m1, in_values=w_t, imm_value=0.0,
                )
                nc.vector.max(m2, w1z)
                # exact same-side compare: keeps exactly the top-16 per row
                nc.vector.tensor_scalar(
                    sel, w_t, m2[:, 7:8], scalar2=None, op0=ALU.is_ge
                )
                nc.vector.tensor_mul(wm_all[:, ds(t * N, N)], w_t, sel)

            # ---- Loop2: transpose masked row-tile (exact), aggregate ----
            with tc.For_i(0, NT, 1) as t:
                nc.sync.dma_start(wmt_t, wm_all[:, ds(t * N, N)])
                for jb in range(NT):
                    tp = ps_mlp.tile([P, P], F16, tag="tp")
                    nc.tensor.transpose(
                        tp, wmt_t[:, jb * P:(jb + 1) * P], identh
                    )
                    nc.scalar.activation(lhs_cols[:, jb, :], tp, AF.Copy)
                for jb in range(NT):
                    nc.tensor.matmul(
                        agg_ps,
                        lhsT=lhs_cols[:, jb, :],
                        rhs=feats_sb[:, jb, :],
                        start=(jb == 0), stop=(jb == NT - 1),
                    )
                nc.vector.reciprocal(recip, agg_ps[:, DP:DP + 1])
                nc.vector.tensor_scalar_mul(wmf, agg_ps[:, 0:DP], recip)
                # per-node symmetric int8 quant: s = absmax/126.5 (margin so
                # |q| <= 127 survives the fp16 rounding of s), downloadable
                # scale is the fp16-rounded s, and q uses 1/that so host
                # dequant q*s reproduces wmean exactly up to the int rounding
                nc.vector.reduce_max(
                    amax, wmf, axis=mybir.AxisListType.X,
                    apply_absolute_value=True,
                )
                nc.vector.tensor_scalar_max(amax, amax, 1e-12)
                nc.scalar.activation(
                    s_all[:, ds(t, 1)], amax, AF.Copy, scale=1.0 / 126.5
                )
                nc.scalar.activation(s32, s_all[:, ds(t, 1)], AF.Copy)
                nc.vector.reciprocal(rq, s32)
                nc.vector.tensor_scalar_mul(
                    o8_all[:, ds(t * DP, DP)], wmf, rq
                )

            nc.sync.dma_start(
                out8_d[0:N, :].rearrange("(t p) d -> p t d", p=P),
                o8_all.rearrange("p (t d) -> p t d", t=NT),
            )  # quantized wmean; the 2-layer MLP runs on the host
            nc.sync.dma_start(
                out8_d[N:N + 64, :].rearrange("p2 (a c) -> (p2 a) c", a=2),
                s_all.bitcast(dt.int8),
            )
            if debug:
                nc.sync.dma_start(dbg_w[:, :], wm_all[:, 0:N])
                nc.sync.dma_start(
                    dbg_lhs[:, :],
                    lhs_cols.rearrange("p j i -> p (j i)"),
                )

    return nc


_CACHE = {}


def _get_nc():
    if "nc" not in _CACHE:
        nc = bacc_mod.Bacc()
        build_gravnet(nc)
        nc.finalize()
        _CACHE["nc"] = nc
    return _CACHE["nc"]


class _Exec:
    """Cached SPMD dispatcher: one jitted shard_map over 8 cores.

    Mirrors what run_bass_kernel_spmd -> bass2jax.run_bass_via_pjrt does
    under axon, but builds the jit wrapper (and the device-side zero-buffer
    creator) exactly once, so warm calls pay no re-trace / persistent-cache
    lookup and no H2D upload of donated output buffers.
    """

    def __init__(self, nc):
        bass2jax.install_neuronx_cc_hook()
        self.nc = nc
        partition_name = (
            nc.partition_id_tensor.name if nc.partition_id_tensor else None
        )
        in_names, out_names, out_avals = [], [], []
        self.out_shapes = []
        for alloc in nc.m.functions[0].allocations:
            if not isinstance(alloc, mybir.MemoryLocationSet):
                continue
            name = alloc.memorylocations[0].name
            if alloc.kind == "ExternalInput":
                if name != partition_name:
                    in_names.append(name)
            elif alloc.kind == "ExternalOutput":
                shape = tuple(alloc.tensor_shape)
                np_dt = mybir.dt.np(alloc.dtype)
                out_names.append(name)
                out_avals.append(jax.core.ShapedArray(shape, np_dt))
                self.out_shapes.append((shape, np_dt))
        assert in_names == ["pk"] and out_names == ["out8"], (
            in_names, out_names,
        )
        n_params = len(in_names)
        n_outs = len(out_avals)
        in_names_full = in_names + out_names
        if partition_name is not None:
            in_names_full.append(partition_name)
        donate = tuple(range(n_params, n_params + n_outs))

        def _body(*args):
            operands = list(args)
            if partition_name is not None:
                operands.append(bass2jax.partition_id_tensor())
            outs = bass2jax._bass_exec_p.bind(
                *operands,
                out_avals=tuple(out_avals),
                in_names=tuple(in_names_full),
                out_names=tuple(out_names),
                lowering_input_output_aliases=(),
                sim_require_finite=True,
                sim_require_nnan=True,
                nc=nc,
            )
            return tuple(outs)

        devices = jax.devices()[:B]
        assert len(devices) == B, f"need {B} devices, got {len(jax.devices())}"
        self.mesh = Mesh(np.asarray(devices), ("core",))
        spec = PartitionSpec("core")
        self.sharding = NamedSharding(self.mesh, spec)
        in_specs = (spec,) * (n_params + n_outs)
        out_specs = (spec,) * n_outs
        self.sharded = jax.jit(
            shard_map(
                _body,
                mesh=self.mesh,
                in_specs=in_specs,
                out_specs=out_specs,
                check_rep=False,
            ),
            donate_argnums=donate,
            keep_unused=True,
        )
        # Device-side creation of the donated output buffers (the bass kernel
        # writes every element of both outputs, so contents are irrelevant,
        # but the custom call needs committed buffers to consume).
        gshapes = [((B * s[0],) + s[1:], d) for s, d in self.out_shapes]
        self.zeros = jax.jit(
            lambda: tuple(jnp.zeros(gs, gd) for gs, gd in gshapes),
            out_shardings=(self.sharding,) * n_outs,
        )
        self._next_zeros = None

    def put(self, arr: np.ndarray):
        """Async upload of a (B*rows, ...) host array, sharded over cores."""
        return jax.device_put(arr, self.sharding)

    def run(self, *in_devs):
        """Dispatch and return per-output lists of per-core shards in core
        order, with async D2H copies already issued (single round trip)."""
        # donated buffers are input-independent: use the set pre-dispatched
        # at the end of the previous call when available
        zs = self._next_zeros if self._next_zeros is not None else self.zeros()
        outs = self.sharded(*in_devs, *zs)
        # pre-dispatch (async, device-side) the next call's donated buffers
        self._next_zeros = self.zeros()
        all_shards = []
        for og in outs:
            shards = sorted(
                og.addressable_shards, key=lambda s: s.index[0].start or 0
            )
            all_shards.append([s.data for s in shards])
        for datas in all_shards:
            for d in datas:
                d.copy_to_host_async()
        return all_shards


def _get_exec():
    if "exec" not in _CACHE:
        _CACHE["exec"] = _Exec(_get_nc())
    return _CACHE["exec"]


def _pack_coords(coords, s16):
    """Packed fp16 coords input [B*RC_END, 64] (view of a cached buffer).

    d2 expansion uses an fp16 hi/lo split of coords (and |c|^2) so the PE
    contraction (exact fp16 products, f32 accumulate) reproduces f32-accurate
    s = -d2:  s = sum_r 2(hi+lo)_i (hi+lo)_j - n2_i - n2_j, dropping lo*lo.
    """
    n2 = np.sum(coords * coords, axis=-1)                # [B,N]
    c_hi = coords.astype(np.float16).astype(np.float32)
    c_lo = coords - c_hi
    n2_hi = n2.astype(np.float16).astype(np.float32)
    n2_lo = n2 - n2_hi
    cT_hi = c_hi.transpose(0, 2, 1)                      # [B,4,N]
    cT_lo = c_lo.transpose(0, 2, 1)

    buf = _CACHE.get("pkc_all")
    if buf is None:
        buf = np.empty((B, RC_END, 64), np.float16)
        _CACHE["pkc_all"] = buf
    pkc = buf
    pkc[:, R_HI:R_HI + 128] = cT_hi.reshape(B, 128, 64)
    pkc[:, R_LO:R_LO + 128] = cT_lo.reshape(B, 128, 64)
    n2n = pkc[:, R_N2:R_N2 + 64].reshape(B, 2, N)        # view
    n2n[:, 0] = -n2_hi
    n2n[:, 1] = -n2_lo
    pkc[:, R_FS:R_FS + 32] = (
        s16.reshape(B, NT, P).transpose(0, 2, 1).reshape(B, 32, 64)
    )
    return pkc.reshape(B * RC_END, 64)


def kernel(**inputs) -> np.ndarray:
    x = np.asarray(inputs["x"], dtype=np.float32)
    mask = np.asarray(inputs["mask"])
    W_space = np.asarray(inputs["W_space"], np.float32)
    b_space = np.asarray(inputs["b_space"], np.float32)
    W_feat = np.asarray(inputs["W_feat"], np.float32)
    b_feat = np.asarray(inputs["b_feat"], np.float32)
    W1 = np.asarray(inputs["W1"], np.float32)
    W2 = np.asarray(inputs["W2"], np.float32)
    b1 = np.asarray(inputs["b1"], np.float32)
    b2 = np.asarray(inputs["b2"], np.float32)

    ex = _get_exec()
    xf = x.reshape(B * N, DIN)

    # feats first: its uint8 copy is most of the upload bytes, so ship it the
    # moment it exists (device_put is async -- serialization+network stream
    # while the host packs coords below). Per-node symmetric quantization:
    # u = round(f/s) + 128 with s = absmax/126.5 rounded to fp16 (the device
    # dequantizes with that same fp16 s, so |u-128| <= 127 is guaranteed).
    # The +128.5 shift makes uint8 truncation equal round-to-nearest.
    bufs = _CACHE.get("host_bufs")
    if bufs is None:
        bufs = {
            "fq": np.empty((B * N, DP), np.float32),
            "pk": np.empty((B, N + 2 * RC_END, DP), np.uint8),
            "h": np.empty((B * N, DOUT), np.float32),
            "out": np.empty((B, N, DOUT), np.float32),
            "wm32": np.empty((N, DP), np.float32),
        }
        _CACHE["host_bufs"] = bufs
    fq, pk = bufs["fq"], bufs["pk"]

    feats = xf @ W_feat                                  # [B*N, DP] f32
    if b_feat.any():
        feats += b_feat
    if not mask.all():
        feats *= mask.reshape(B * N, 1)
    s = feats.max(axis=1)
    smin = feats.min(axis=1)
    np.negative(smin, out=smin)
    np.maximum(s, smin, out=s)
    np.maximum(s, 1e-12, out=s)
    s /= 126.5
    s16 = s.astype(np.float16)                           # [B*N]
    s32 = s16.astype(np.float32)
    np.multiply(feats, (1.0 / s32)[:, None], out=fq)
    fq += 128.5
    # trunc of (v+128.5) == round(v)+128; NB pk[:, 0:N] is a strided view,
    # copyto writes through it (a .reshape here would silently copy)
    np.copyto(pk[:, 0:N], fq.reshape(B, N, DP), casting="unsafe")

    coords = (xf @ (W_space * CSCALE)).reshape(B, N, DS)
    if b_space.any():
        coords += b_space * CSCALE
    pkc = _pack_coords(coords, s16).reshape(B, RC_END * 64)
    pk[:, N:].reshape(B, -1)[:] = pkc.view(np.uint8)

    (q_shards,) = ex.run(pk.reshape(B * (N + 2 * RC_END), DP))

    # MLP: out = relu([feats|wmean] @ W1 + b1) @ W2 + b2. The feats half
    # doesn't need device results -- compute it while the tunnel round trip
    # is in flight, then pipeline the per-batch tail against the per-shard
    # download stream (network-bound, so the CPU is free).
    W1a, W1b = W1[:DP], W1[DP:]
    h = bufs["h"]
    np.matmul(feats, W1a, out=h)
    h += b1
    h = h.reshape(B, N, DOUT)
    out, wm32 = bufs["out"], bufs["wm32"]
    for b in range(B):
        sh = np.asarray(q_shards[b])                     # [N+64, DP] int8
        np.copyto(wm32, sh[0:N], casting="unsafe")
        ws = sh[N:].reshape(-1).view(np.float16).reshape(P, NT)
        wm32 *= ws.astype(np.float32).T.reshape(N, 1)    # node n = t*P + p
        hb = h[b]
        hb += wm32 @ W1b
        np.maximum(hb, 0.0, out=hb)
        np.matmul(hb, W2, out=out[b])
        out[b] += b2
    return out


if __name__ == "__main__":
    rng = np.random.default_rng(0)
    ins = {
        "x": rng.standard_normal((8, N, DIN), dtype=np.float32),
        "mask": np.ones((8, N), bool),
        "W_space": rng.standard_normal((DIN, DS), dtype=np.float32) * 0.02,
        "b_space": np.zeros(DS, np.float32),
        "W_feat": rng.standard_normal((DIN, DP), dtype=np.float32) * 0.02,
        "b_feat": np.zeros(DP, np.float32),
        "W1": rng.standard_normal((2 * DP, DOUT), dtype=np.float32) * 0.02,
        "b1": np.zeros(DOUT, np.float32),
        "W2": rng.standard_normal((DOUT, DOUT), dtype=np.float32) * 0.02,
        "b2": np.zeros(DOUT, np.float32),
    }
    print(kernel(**ins).shape)


# revision 56
# speedup vs baseline: 1.0343x; 1.0343x over previous
"""GravNet layer Bass kernel for Trainium2, 8 NeuronCores (data-parallel over batch).

Wall time through the axon tunnel is dominated by tunnel round-trip latency
(~75-110ms per blocking sync) plus per-MB transfer cost (~10-20ms/MB), not
device compute (<1ms). This version is built around one round trip per call
and minimum bytes each way:

  * The jax.jit(shard_map(bass_exec)) wrapper is built ONCE and cached --
    run_bass_kernel_spmd re-creates it every call, paying a re-trace +
    persistent-cache lookup (~25ms) per call.
  * ONE merged input tensor per core (uint8-quantized feats + fp16 coords
    pack bit-cast into byte rows) -> a single device_put; ONE merged output
    tensor (int8-quantized wmean + its fp16 scales bit-cast into extra
    rows) -> 8 fetches instead of 16+ per call.
  * Donated output buffers are created ON DEVICE by a cached jitted zeros fn
    pre-dispatched at the END of the previous call -- no H2D upload, no
    dispatch latency on the current call.
  * Output shards are fetched with copy_to_host_async issued immediately
    after dispatch: wait-for-ready and D2H collapse into a single round trip
    (block_until_ready + asarray would be two).
  * Quantization: feats ride as uint8 u = round(f/s)+128 with a per-node
    fp16 scale s = absmax/126.5 (the +128.5 host trick makes uint8
    truncation equal round-to-nearest); wmean returns as per-node-scaled
    int8 the same way. Both quantizations together cost 2.3e-3 final rel
    err vs the 2e-2 gate (the host MLP input `feats` stays exact f32; only
    the neighbor-aggregation payload is quantized).
  * The host half of the MLP that doesn't need device results
    (feats @ W1[:64] + b1) runs during the tunnel wait, and the per-batch
    MLP tail (wmean @ W1[64:], relu, @W2) is pipelined against the
    per-shard download stream.

Host (~0.1% of FLOPs): coords = x@W_space, feats = x@W_feat, quantization,
and the final 2-layer MLP in f32 (more accurate than a device fp16 MLP and
cheaper than downloading a 128-wide result). The d2 expansion uses an fp16
hi/lo split of coords and |c|^2 over a 16-row contraction (2 hi*hi + 2 hi*lo
+ 2 lo*hi - n2 terms), so the PE's exact fp16 products + f32 PSUM
accumulation reproduce s = -d2 to ~1e-6 -- fp32 PE matmul (fp32r) and plain
fp16 coords both lose enough precision to flip kNN selections vs the
reference (~1e-2 rel err). Coords are pre-scaled by CSCALE so the hi/lo
residuals stay out of fp16-subnormal range (slow numpy conversions).

Device (per core, one batch element):
  Staging: dequantize feats (u-128)*s -> fp16 [feats|1]; build A/B
      expansion rows from the 10 uploaded coord rows.
  Loop1 (t in 16): s row-tile via matmul, w = exp(10/CSCALE^2 * s) in f32;
      top-8 twice (max8 + match_replace + max8) then an exact same-side
      compare w >= m2[:,7] keeps exactly the row-wise top-16 (f32, no
      ties); masked weights stored fp16.
  Loop2 (t in 16): PE-transpose the 16 blocks of the masked row-tile (exact
      for fp16 values) -> lhsT; aggregate against [feats|1] with PSUM
      accumulation; weighted mean -> per-node int8 quant + fp16 scale.
Output tile t needs exactly the transposed blocks of masked row-tile t, so
there is no index gather anywhere. Biases b1/b2 are applied on the host;
mask zeroes feats on the host (all-ones in this problem's spec).
"""

# Persistent XLA compilation cache so a cold process only pays neuronx-cc
# once per executable across runs. jax may already be initialized by the
# site hook, so set via config.update, not env vars.
import jax

jax.config.update("jax_compilation_cache_dir", "/tmp/jax_comp_cache")
jax.config.update("jax_persistent_cache_min_compile_time_secs", 0.0)
jax.config.update("jax_persistent_cache_min_entry_size_bytes", 0)

import numpy as np
import jax.numpy as jnp
from jax.sharding import Mesh, NamedSharding, PartitionSpec

import concourse.bass as bass
import concourse.bacc as bacc_mod
import concourse.bass2jax as bass2jax
import concourse.mybir as mybir
import concourse.tile as tile
from concourse.bass import ds
from concourse.masks import make_identity

# the deprecated experimental entry point still accepts check_rep (it's what
# bass2jax itself uses); jax.shard_map renamed it to check_vma
from jax.experimental.shard_map import shard_map

P = 128
N = 2048
DIN = 128
DS = 4
DP = 64
DOUT = 128
NT = N // P          # 16 row tiles
FREE = 512
JC = N // FREE       # 4 psum-bank chunks
B = 8
dt = mybir.dt
AF = mybir.ActivationFunctionType
ALU = mybir.AluOpType
F16 = dt.float16
F32 = dt.float32

# Coords are scaled by CSCALE (a power of two) before the fp16 hi/lo split:
# kNN ordering is scale-invariant and the device folds 1/CSCALE^2 into the
# exp() scale, but the split residuals move from ~2e-5 (fp16 SUBNORMAL --
# numpy's slow conversion path, ~3.5ms per call) to ~1.3e-3 (normal, fast).
# 64 keeps n2*CSCALE^2 (max ~6.5e3) well inside fp16 range.
CSCALE = 64.0

# packed fp16 coords-input rows (width 64)
R_HI = 0                   # [128, 64]   coords hi  [4, 2048]
R_LO = R_HI + 128          # [128, 64]   coords lo  [4, 2048]
R_N2 = R_LO + 128          # [64, 64]    [-n2_hi; -n2_lo] [2, 2048]
R_FS = R_N2 + 64           # [32, 64]    feat scales fp16 [P, NT] row-major
RC_END = R_FS + 32         # 352


def build_gravnet(nc: bass.Bass, debug: bool = False):
    # single merged input: N rows of uint8 feats + 2*RC_END rows carrying the
    # fp16 coords pack bit-cast to byte pairs (one device_put per call)
    pk_d = nc.dram_tensor("pk", [N + 2 * RC_END, DP], dt.uint8, kind="ExternalInput")
    pkf_d = pk_d[0:N, :]
    pk16_d = pk_d.bitcast(F16)          # [N + 2*RC_END, 32] fp16 view

    def pkc_view(row, n_rows):
        """fp16 view [2*n_rows, 32] of pkc-layout rows [row, row+n_rows)."""
        return pk16_d[N + 2 * row:N + 2 * (row + n_rows), :]
    # single output: N rows of int8 wmean + 64 rows carrying the per-node
    # fp16 scales bit-cast to int8 pairs (merging them into one tensor saves
    # 8 extra per-device fetch RPCs through the tunnel)
    out8_d = nc.dram_tensor("out8", [N + 64, DP], dt.int8, kind="ExternalOutput")
    if debug:
        dbg_w = nc.dram_tensor("dbg_w", [P, N], F16, kind="ExternalOutput")
        dbg_lhs = nc.dram_tensor("dbg_lhs", [P, NT * P], F16, kind="ExternalOutput")

    with tile.TileContext(nc) as tc:
        with (
            tc.tile_pool(name="big", bufs=1) as big,
            tc.tile_pool(name="ps_s", bufs=1, space="PSUM") as ps_s,
            tc.tile_pool(name="ps_agg", bufs=1, space="PSUM") as ps_agg,
            tc.tile_pool(name="ps_mlp", bufs=1, space="PSUM") as ps_mlp,
        ):
            # ---- constants / staged inputs ----
            identh = big.tile([P, P], F16)
            make_identity(nc, identh)

            f8_sb = big.tile([P, NT, DP], dt.uint8)
            nc.sync.dma_start(
                f8_sb, pkf_d[:, :].rearrange("(t p) d -> p t d", p=P)
            )
            # feat scales ride in pkc: [P, NT] fp16 row-major = [32, 64] rows;
            # element (p, t) sits at dram (p//4, 16*(p%4)+t)
            fs16_sb = big.tile([P, NT], F16)
            nc.sync.dma_start(
                fs16_sb,
                pkc_view(R_FS, 32).rearrange(
                    "(r pb1) (pb0 t) -> (r pb1 pb0) t", pb1=2, pb0=2
                ),
            )
            fs_sb = big.tile([P, NT], F32)
            nc.vector.tensor_copy(fs_sb, fs16_sb)
            feats_sb = big.tile([P, NT, DP + 1], F16)
            nc.vector.memset(feats_sb[:, :, DP:DP + 1], 1.0)
            # dequant: feats = (u - 128) * s_node, one fused op per tile
            for t in range(NT):
                nc.vector.tensor_scalar(
                    feats_sb[:, t, 0:DP],
                    f8_sb[:, t, :],
                    -128.0,
                    scalar2=fs_sb[:, t:t + 1],
                    op0=ALU.add,
                    op1=ALU.mult,
                )

            # A/B expansion rows rebuilt on device from the minimal 10 rows:
            # A = [hi hi lo -n2hi -n2lo 1 1], B = [2hi 2lo 2hi 1 1 -n2hi -n2lo]
            hi_t = big.tile([4, N], F16)
            nc.sync.dma_start(
                hi_t,
                pkc_view(R_HI, 128).rearrange(
                    "(r jh jl1) jl0 -> r (jh jl1 jl0)", r=4, jh=32, jl1=2
                ),
            )
            lo_t = big.tile([4, N], F16)
            nc.sync.dma_start(
                lo_t,
                pkc_view(R_LO, 128).rearrange(
                    "(r jh jl1) jl0 -> r (jh jl1 jl0)", r=4, jh=32, jl1=2
                ),
            )
            n2n_t = big.tile([2, N], F16)
            nc.sync.dma_start(
                n2n_t,
                pkc_view(R_N2, 64).rearrange(
                    "(r jh jl1) jl0 -> r (jh jl1 jl0)", r=2, jh=32, jl1=2
                ),
            )
            hi2_t = big.tile([4, N], F16)
            nc.scalar.activation(hi2_t, hi_t, AF.Copy, scale=2.0)  # exact in fp16
            lo2_t = big.tile([4, N], F16)
            nc.scalar.activation(lo2_t, lo_t, AF.Copy, scale=2.0)
            ones2 = big.tile([2, N], F16)
            nc.vector.memset(ones2, 1.0)
            A_sb = big.tile([16, N], F16)
            B_sb = big.tile([16, N], F16)
            nc.sync.dma_start(A_sb[0:4, :], hi_t)
            nc.sync.dma_start(A_sb[4:8, :], hi_t)
            nc.sync.dma_start(A_sb[8:12, :], lo_t)
            nc.sync.dma_start(A_sb[12:14, :], n2n_t)
            nc.sync.dma_start(A_sb[14:16, :], ones2)
            nc.sync.dma_start(B_sb[0:4, :], hi2_t)
            nc.sync.dma_start(B_sb[4:8, :], lo2_t)
            nc.sync.dma_start(B_sb[8:12, :], hi2_t)
            nc.sync.dma_start(B_sb[12:14, :], ones2)
            nc.sync.dma_start(B_sb[14:16, :], n2n_t)

            # ---- persistent state ----
            wm_all = big.tile([P, NT * N], F16)    # masked weight row-tiles
            o8_all = big.tile([P, NT * DP], dt.int8)   # quantized wmean tiles
            s_all = big.tile([P, NT], F16)             # per-node wmean scales

            # scratch (fixed addresses; loop back-edge serializes iterations)
            a_t = big.tile([16, P], F16)
            w_t = big.tile([P, N], F32)
            m1 = big.tile([P, 8], F32)
            m2 = big.tile([P, 8], F32)
            w1z = big.tile([P, N], F32)
            sel = big.tile([P, N], F32)
            wmt_t = big.tile([P, N], F16)
            lhs_cols = big.tile([P, NT, P], F16)   # transposed masked weights
            recip = big.tile([P, 1], F32)
            wmf = big.tile([P, DP], F32)
            amax = big.tile([P, 1], F32)
            s32 = big.tile([P, 1], F32)
            rq = big.tile([P, 1], F32)

            s_ps = ps_s.tile([P, N], F32)                  # 4 banks
            agg_ps = ps_agg.tile([P, DP + 1], F32)         # 1 bank

            # ---- Loop1: distances, exp, exact row-side top-16 mask ----
            with tc.For_i(0, NT, 1) as t:
                nc.sync.dma_start(a_t, A_sb[:, ds(t * P, P)])
                for c in range(JC):
                    nc.tensor.matmul(
                        s_ps[:, c * FREE:(c + 1) * FREE],
                        lhsT=a_t,
                        rhs=B_sb[:, c * FREE:(c + 1) * FREE],
                        start=True, stop=True,
                    )
                for c in range(JC):
                    nc.scalar.activation(
                        w_t[:, c * FREE:(c + 1) * FREE],
                        s_ps[:, c * FREE:(c + 1) * FREE],
                        AF.Exp, scale=10.0 / (CSCALE * CSCALE),
                    )
                nc.vector.max(m1, w_t)
                nc.vector.match_replace(
                    w1z, in_to_replace=m1, in_values=w_t, imm_value=0.0,
                )
                nc.vector.max(m2, w1z)
                # exact same-side compare: keeps exactly the top-16 per row
                nc.vector.tensor_scalar(
                    sel, w_t, m2[:, 7:8], scalar2=None, op0=ALU.is_ge
                )
                nc.vector.tensor_mul(wm_all[:, ds(t * N, N)], w_t, sel)

            # ---- Loop2: transpose masked row-tile (exact), aggregate ----
            with tc.For_i(0, NT, 1) as t:
                nc.sync.dma_start(wmt_t, wm_all[:, ds(t * N, N)])
                for jb in range(NT):
                    tp = ps_mlp.tile([P, P], F16, tag="tp")
                    nc.tensor.transpose(
                        tp, wmt_t[:, jb * P:(jb + 1) * P], identh
                    )
                    nc.scalar.activation(lhs_cols[:, jb, :], tp, AF.Copy)
                for jb in range(NT):
                    nc.tensor.matmul(
                        agg_ps,
                        lhsT=lhs_cols[:, jb, :],
                        rhs=feats_sb[:, jb, :],
                        start=(jb == 0), stop=(jb == NT - 1),
                    )
                nc.vector.reciprocal(recip, agg_ps[:, DP:DP + 1])
                nc.vector.tensor_scalar_mul(wmf, agg_ps[:, 0:DP], recip)
                # per-node symmetric int8 quant: s = absmax/126.5 (margin so
                # |q| <= 127 survives the fp16 rounding of s), downloadable
                # scale is the fp16-rounded s, and q uses 1/that so host
                # dequant q*s reproduces wmean exactly up to the int rounding
                nc.vector.reduce_max(
                    amax, wmf, axis=mybir.AxisListType.X,
                    apply_absolute_value=True,
                )
                nc.vector.tensor_scalar_max(amax, amax, 1e-12)
                nc.scalar.activation(
                    s_all[:, ds(t, 1)], amax, AF.Copy, scale=1.0 / 126.5
                )
                nc.scalar.activation(s32, s_all[:, ds(t, 1)], AF.Copy)
                nc.vector.reciprocal(rq, s32)
                nc.vector.tensor_scalar_mul(
                    o8_all[:, ds(t * DP, DP)], wmf, rq
                )

            nc.sync.dma_start(
                out8_d[0:N, :].rearrange("(t p) d -> p t d", p=P),
                o8_all.rearrange("p (t d) -> p t d", t=NT),
            )  # quantized wmean; the 2-layer MLP runs on the host
            nc.sync.dma_start(
                out8_d[N:N + 64, :].rearrange("p2 (a c) -> (p2 a) c", a=2),
                s_all.bitcast(dt.int8),
            )
            if debug:
                nc.sync.dma_start(dbg_w[:, :], wm_all[:, 0:N])
                nc.sync.dma_start(
                    dbg_lhs[:, :],
                    lhs_cols.rearrange("p j i -> p (j i)"),
                )

    return nc


_CACHE = {}


def _get_nc():
    if "nc" not in _CACHE:
        nc = bacc_mod.Bacc()
        build_gravnet(nc)
        nc.finalize()
        _CACHE["nc"] = nc
    return _CACHE["nc"]


class _Exec:
    """Cached SPMD dispatcher: one jitted shard_map over 8 cores.

    Mirrors what run_bass_kernel_spmd -> bass2jax.run_bass_via_pjrt does
    under axon, but builds the jit wrapper (and the device-side zero-buffer
    creator) exactly once, so warm calls pay no re-trace / persistent-cache
    lookup and no H2D upload of donated output buffers.
    """

    def __init__(self, nc):
        bass2jax.install_neuronx_cc_hook()
        self.nc = nc
        partition_name = (
            nc.partition_id_tensor.name if nc.partition_id_tensor else None
        )
        in_names, out_names, out_avals = [], [], []
        self.out_shapes = []
        for alloc in nc.m.functions[0].allocations:
            if not isinstance(alloc, mybir.MemoryLocationSet):
                continue
            name = alloc.memorylocations[0].name
            if alloc.kind == "ExternalInput":
                if name != partition_name:
                    in_names.append(name)
            elif alloc.kind == "ExternalOutput":
                shape = tuple(alloc.tensor_shape)
                np_dt = mybir.dt.np(alloc.dtype)
                out_names.append(name)
                out_avals.append(jax.core.ShapedArray(shape, np_dt))
                self.out_shapes.append((shape, np_dt))
        assert in_names == ["pk"] and out_names == ["out8"], (
            in_names, out_names,
        )
        n_params = len(in_names)
        n_outs = len(out_avals)
        in_names_full = in_names + out_names
        if partition_name is not None:
            in_names_full.append(partition_name)
        donate = tuple(range(n_params, n_params + n_outs))

        def _body(*args):
            operands = list(args)
            if partition_name is not None:
                operands.append(bass2jax.partition_id_tensor())
            outs = bass2jax._bass_exec_p.bind(
                *operands,
                out_avals=tuple(out_avals),
                in_names=tuple(in_names_full),
                out_names=tuple(out_names),
                lowering_input_output_aliases=(),
                sim_require_finite=True,
                sim_require_nnan=True,
                nc=nc,
            )
            return tuple(outs)

        devices = jax.devices()[:B]
        assert len(devices) == B, f"need {B} devices, got {len(jax.devices())}"
        self.mesh = Mesh(np.asarray(devices), ("core",))
        spec = PartitionSpec("core")
        self.sharding = NamedSharding(self.mesh, spec)
        in_specs = (spec,) * (n_params + n_outs)
        out_specs = (spec,) * n_outs
        self.sharded = jax.jit(
            shard_map(
                _body,
                mesh=self.mesh,
                in_specs=in_specs,
                out_specs=out_specs,
                check_rep=False,
            ),
            donate_argnums=donate,
            keep_unused=True,
        )
        # Device-side creation of the donated output buffers (the bass kernel
        # writes every element of both outputs, so contents are irrelevant,
        # but the custom call needs committed buffers to consume).
        gshapes = [((B * s[0],) + s[1:], d) for s, d in self.out_shapes]
        self.zeros = jax.jit(
            lambda: tuple(jnp.zeros(gs, gd) for gs, gd in gshapes),
            out_shardings=(self.sharding,) * n_outs,
        )
        self._next_zeros = None

    def put(self, arr: np.ndarray):
        """Async upload of a (B*rows, ...) host array, sharded over cores."""
        return jax.device_put(arr, self.sharding)

    def run(self, *in_devs):
        """Dispatch and return per-output lists of per-core shards in core
        order, with async D2H copies already issued (single round trip)."""
        # donated buffers are input-independent: use the set pre-dispatched
        # at the end of the previous call when available
        zs = self._next_zeros if self._next_zeros is not None else self.zeros()
        outs = self.sharded(*in_devs, *zs)
        # pre-dispatch (async, device-side) the next call's donated buffers
        self._next_zeros = self.zeros()
        all_shards = []
        for og in outs:
            shards = sorted(
                og.addressable_shards, key=lambda s: s.index[0].start or 0
            )
            all_shards.append([s.data for s in shards])
        for datas in all_shards:
            for d in datas:
                d.copy_to_host_async()
        return all_shards


def _get_exec():
    if "exec" not in _CACHE:
        _CACHE["exec"] = _Exec(_get_nc())
    return _CACHE["exec"]


def _pack_coords(coords, s16):
    """Packed fp16 coords input [B*RC_END, 64] (view of a cached buffer).

    d2 expansion uses an fp16 hi/lo split of coords (and |c|^2) so the PE
    contraction (exact fp16 products, f32 accumulate) reproduces f32-accurate
    s = -d2:  s = sum_r 2(hi+lo)_i (hi+lo)_j - n2_i - n2_j, dropping lo*lo.
    """
    n2 = np.sum(coords * coords, axis=-1)                # [B,N]
    c_hi = coords.astype(np.float16).astype(np.float32)
    c_lo = coords - c_hi
    n2_hi = n2.astype(np.float16).astype(np.float32)
    n2_lo = n2 - n2_hi
    cT_hi = c_hi.transpose(0, 2, 1)                      # [B,4,N]
    cT_lo = c_lo.transpose(0, 2, 1)

    buf = _CACHE.get("pkc_all")
    if buf is None:
        buf = np.empty((B, RC_END, 64), np.float16)
        _CACHE["pkc_all"] = buf
    pkc = buf
    pkc[:, R_HI:R_HI + 128] = cT_hi.reshape(B, 128, 64)
    pkc[:, R_LO:R_LO + 128] = cT_lo.reshape(B, 128, 64)
    n2n = pkc[:, R_N2:R_N2 + 64].reshape(B, 2, N)        # view
    n2n[:, 0] = -n2_hi
    n2n[:, 1] = -n2_lo
    pkc[:, R_FS:R_FS + 32] = (
        s16.reshape(B, NT, P).transpose(0, 2, 1).reshape(B, 32, 64)
    )
    return pkc.reshape(B * RC_END, 64)


def kernel(**inputs) -> np.ndarray:
    x = np.asarray(inputs["x"], dtype=np.float32)
    mask = np.asarray(inputs["mask"])
    W_space = np.asarray(inputs["W_space"], np.float32)
    b_space = np.asarray(inputs["b_space"], np.float32)
    W_feat = np.asarray(inputs["W_feat"], np.float32)
    b_feat = np.asarray(inputs["b_feat"], np.float32)
    W1 = np.asarray(inputs["W1"], np.float32)
    W2 = np.asarray(inputs["W2"], np.float32)
    b1 = np.asarray(inputs["b1"], np.float32)
    b2 = np.asarray(inputs["b2"], np.float32)

    ex = _get_exec()
    xf = x.reshape(B * N, DIN)

    # feats first: its uint8 copy is most of the upload bytes, so ship it the
    # moment it exists (device_put is async -- serialization+network stream
    # while the host packs coords below). Per-node symmetric quantization:
    # u = round(f/s) + 128 with s = absmax/126.5 rounded to fp16 (the device
    # dequantizes with that same fp16 s, so |u-128| <= 127 is guaranteed).
    # The +128.5 shift makes uint8 truncation equal round-to-nearest.
    bufs = _CACHE.get("host_bufs")
    if bufs is None:
        bufs = {
            "fq": np.empty((B * N, DP), np.float32),
            "pk": np.empty((B, N + 2 * RC_END, DP), np.uint8),
            "h": np.empty((B * N, DOUT), np.float32),
            "out": np.empty((B, N, DOUT), np.float32),
            "wm32": np.empty((N, DP), np.float32),
        }
        _CACHE["host_bufs"] = bufs
    fq, pk = bufs["fq"], bufs["pk"]

    feats = xf @ W_feat                                  # [B*N, DP] f32
    if b_feat.any():
        feats += b_feat
    if not mask.all():
        feats *= mask.reshape(B * N, 1)
    s = feats.max(axis=1)
    smin = feats.min(axis=1)
    np.negative(smin, out=smin)
    np.maximum(s, smin, out=s)
    np.maximum(s, 1e-12, out=s)
    s /= 126.5
    s16 = s.astype(np.float16)                           # [B*N]
    s32 = s16.astype(np.float32)
    np.multiply(feats, (1.0 / s32)[:, None], out=fq)
    fq += 128.5
    # trunc of (v+128.5) == round(v)+128; NB pk[:, 0:N] is a strided view,
    # copyto writes through it (a .reshape here would silently copy)
    np.copyto(pk[:, 0:N], fq.reshape(B, N, DP), casting="unsafe")

    coords = (xf @ (W_space * CSCALE)).reshape(B, N, DS)
    if b_space.any():
        coords += b_space * CSCALE
    pkc = _pack_coords(coords, s16).reshape(B, RC_END * 64)
    pk[:, N:].reshape(B, -1)[:] = pkc.view(np.uint8)

    pk_dev = ex.put(pk.reshape(B * (N + 2 * RC_END), DP))
    (q_shards,) = ex.run(pk_dev)

    # MLP: out = relu([feats|wmean] @ W1 + b1) @ W2 + b2. The feats half
    # doesn't need device results -- compute it while the tunnel round trip
    # is in flight, then pipeline the per-batch tail against the per-shard
    # download stream (network-bound, so the CPU is free).
    W1a, W1b = W1[:DP], W1[DP:]
    h = bufs["h"]
    np.matmul(feats, W1a, out=h)
    h += b1
    h = h.reshape(B, N, DOUT)
    out, wm32 = bufs["out"], bufs["wm32"]
    for b in range(B):
        sh = np.asarray(q_shards[b])                     # [N+64, DP] int8
        np.copyto(wm32, sh[0:N], casting="unsafe")
        ws = sh[N:].reshape(-1).view(np.float16).reshape(P, NT)
        wm32 *= ws.astype(np.float32).T.reshape(N, 1)    # node n = t*P + p
        hb = h[b]
        hb += wm32 @ W1b
        np.maximum(hb, 0.0, out=hb)
        np.matmul(hb, W2, out=out[b])
        out[b] += b2
    return out


if __name__ == "__main__":
    rng = np.random.default_rng(0)
    ins = {
        "x": rng.standard_normal((8, N, DIN), dtype=np.float32),
        "mask": np.ones((8, N), bool),
        "W_space": rng.standard_normal((DIN, DS), dtype=np.float32) * 0.02,
        "b_space": np.zeros(DS, np.float32),
        "W_feat": rng.standard_normal((DIN, DP), dtype=np.float32) * 0.02,
        "b_feat": np.zeros(DP, np.float32),
        "W1": rng.standard_normal((2 * DP, DOUT), dtype=np.float32) * 0.02,
        "b1": np.zeros(DOUT, np.float32),
        "W2": rng.standard_normal((DOUT, DOUT), dtype=np.float32) * 0.02,
        "b2": np.zeros(DOUT, np.float32),
    }
    print(kernel(**ins).shape)


# revision 58
# speedup vs baseline: 1.0747x; 1.0391x over previous
"""GravNet layer Bass kernel for Trainium2, 8 NeuronCores (data-parallel over batch).

Wall time through the axon tunnel is dominated by tunnel round-trip latency
(~75-110ms per blocking sync) plus per-MB transfer cost (~10-20ms/MB), not
device compute (<1ms). This version is built around one round trip per call
and minimum bytes each way:

  * The jax.jit(shard_map(bass_exec)) wrapper is built ONCE and cached --
    run_bass_kernel_spmd re-creates it every call, paying a re-trace +
    persistent-cache lookup (~25ms) per call.
  * ONE merged input tensor per core (uint8-quantized feats + fp16 coords
    pack bit-cast into byte rows) -> a single device_put; ONE merged output
    tensor (int8-quantized wmean + its fp16 scales bit-cast into extra
    rows) -> 8 fetches instead of 16+ per call.
  * Donated output buffers are created ON DEVICE by a cached jitted zeros fn
    pre-dispatched at the END of the previous call -- no H2D upload, no
    dispatch latency on the current call.
  * Output shards are fetched with copy_to_host_async issued immediately
    after dispatch: wait-for-ready and D2H collapse into a single round trip
    (block_until_ready + asarray would be two).
  * Quantization: feats ride as uint8 u = round(f/s)+128 with a per-node
    fp16 scale s = absmax/126.5 (the +128.5 host trick makes uint8
    truncation equal round-to-nearest); wmean returns as per-node-scaled
    int8 the same way. Both quantizations together cost 2.3e-3 final rel
    err vs the 2e-2 gate (the host MLP input `feats` stays exact f32; only
    the neighbor-aggregation payload is quantized).
  * The host half of the MLP that doesn't need device results
    (feats @ W1[:64] + b1) runs during the tunnel wait, and the per-batch
    MLP tail (wmean @ W1[64:], relu, @W2) is pipelined against the
    per-shard download stream.

Host (~0.1% of FLOPs): coords = x@W_space, feats = x@W_feat, quantization,
and the final 2-layer MLP in f32 (more accurate than a device fp16 MLP and
cheaper than downloading a 128-wide result). The d2 expansion uses an fp16
hi/lo split of coords and |c|^2 over a 16-row contraction (2 hi*hi + 2 hi*lo
+ 2 lo*hi - n2 terms), so the PE's exact fp16 products + f32 PSUM
accumulation reproduce s = -d2 to ~1e-6 -- fp32 PE matmul (fp32r) and plain
fp16 coords both lose enough precision to flip kNN selections vs the
reference (~1e-2 rel err). Coords are pre-scaled by CSCALE so the hi/lo
residuals stay out of fp16-subnormal range (slow numpy conversions).

Device (per core, one batch element):
  Staging: dequantize feats (u-128)*s -> fp16 [feats|1]; build A/B
      expansion rows from the 10 uploaded coord rows.
  Loop1 (t in 16): s row-tile via matmul, w = exp(10/CSCALE^2 * s) in f32;
      top-8 twice (max8 + match_replace + max8) then an exact same-side
      compare w >= m2[:,7] keeps exactly the row-wise top-16 (f32, no
      ties); masked weights stored fp16.
  Loop2 (t in 16): PE-transpose the 16 blocks of the masked row-tile (exact
      for fp16 values) -> lhsT; aggregate against [feats|1] with PSUM
      accumulation; weighted mean -> per-node int8 quant + fp16 scale.
Output tile t needs exactly the transposed blocks of masked row-tile t, so
there is no index gather anywhere. Biases b1/b2 are applied on the host;
mask zeroes feats on the host (all-ones in this problem's spec).
"""

# Persistent XLA compilation cache so a cold process only pays neuronx-cc
# once per executable across runs. jax may already be initialized by the
# site hook, so set via config.update, not env vars.
import jax

jax.config.update("jax_compilation_cache_dir", "/tmp/jax_comp_cache")
jax.config.update("jax_persistent_cache_min_compile_time_secs", 0.0)
jax.config.update("jax_persistent_cache_min_entry_size_bytes", 0)

import numpy as np
import jax.numpy as jnp
from jax.sharding import Mesh, NamedSharding, PartitionSpec

import concourse.bass as bass
import concourse.bacc as bacc_mod
import concourse.bass2jax as bass2jax
import concourse.mybir as mybir
import concourse.tile as tile
from concourse.bass import ds
from concourse.masks import make_identity

# the deprecated experimental entry point still accepts check_rep (it's what
# bass2jax itself uses); jax.shard_map renamed it to check_vma
from jax.experimental.shard_map import shard_map

P = 128
N = 2048
DIN = 128
DS = 4
DP = 64
DOUT = 128
NT = N // P          # 16 row tiles
FREE = 512
JC = N // FREE       # 4 psum-bank chunks
B = 8
dt = mybir.dt
AF = mybir.ActivationFunctionType
ALU = mybir.AluOpType
F16 = dt.float16
F32 = dt.float32

# Coords are scaled by CSCALE (a power of two) before the fp16 hi/lo split:
# kNN ordering is scale-invariant and the device folds 1/CSCALE^2 into the
# exp() scale, but the split residuals move from ~2e-5 (fp16 SUBNORMAL --
# numpy's slow conversion path, ~3.5ms per call) to ~1.3e-3 (normal, fast).
# 64 keeps n2*CSCALE^2 (max ~6.5e3) well inside fp16 range.
CSCALE = 64.0

# packed fp16 coords-input rows (width 64)
R_HI = 0                   # [128, 64]   coords hi  [4, 2048]
R_LO = R_HI + 128          # [128, 64]   coords lo  [4, 2048]
R_N2 = R_LO + 128          # [64, 64]    [-n2_hi; -n2_lo] [2, 2048]
R_FS = R_N2 + 64           # [32, 64]    feat scales fp16 [P, NT] row-major
RC_END = R_FS + 32         # 352


def build_gravnet(nc: bass.Bass, debug: bool = False):
    # single merged input: N rows of uint8 feats + 2*RC_END rows carrying the
    # fp16 coords pack bit-cast to byte pairs (one device_put per call)
    pk_d = nc.dram_tensor("pk", [N + 2 * RC_END, DP], dt.uint8, kind="ExternalInput")
    pkf_d = pk_d[0:N, :]
    pk16_d = pk_d.bitcast(F16)          # [N + 2*RC_END, 32] fp16 view

    def pkc_view(row, n_rows):
        """fp16 view [2*n_rows, 32] of pkc-layout rows [row, row+n_rows)."""
        return pk16_d[N + 2 * row:N + 2 * (row + n_rows), :]
    # single output: N rows of int8 wmean + 64 rows carrying the per-node
    # fp16 scales bit-cast to int8 pairs (merging them into one tensor saves
    # 8 extra per-device fetch RPCs through the tunnel)
    out8_d = nc.dram_tensor("out8", [N + 64, DP], dt.int8, kind="ExternalOutput")
    if debug:
        dbg_w = nc.dram_tensor("dbg_w", [P, N], F16, kind="ExternalOutput")
        dbg_lhs = nc.dram_tensor("dbg_lhs", [P, NT * P], F16, kind="ExternalOutput")

    with tile.TileContext(nc) as tc:
        with (
            tc.tile_pool(name="big", bufs=1) as big,
            tc.tile_pool(name="ps_s", bufs=1, space="PSUM") as ps_s,
            tc.tile_pool(name="ps_agg", bufs=1, space="PSUM") as ps_agg,
            tc.tile_pool(name="ps_mlp", bufs=1, space="PSUM") as ps_mlp,
        ):
            # ---- constants / staged inputs ----
            identh = big.tile([P, P], F16)
            make_identity(nc, identh)

            f8_sb = big.tile([P, NT, DP], dt.uint8)
            nc.sync.dma_start(
                f8_sb, pkf_d[:, :].rearrange("(t p) d -> p t d", p=P)
            )
            # feat scales ride in pkc: [P, NT] fp16 row-major = [32, 64] rows;
            # element (p, t) sits at dram (p//4, 16*(p%4)+t)
            fs16_sb = big.tile([P, NT], F16)
            nc.sync.dma_start(
                fs16_sb,
                pkc_view(R_FS, 32).rearrange(
                    "(r pb1) (pb0 t) -> (r pb1 pb0) t", pb1=2, pb0=2
                ),
            )
            fs_sb = big.tile([P, NT], F32)
            nc.vector.tensor_copy(fs_sb, fs16_sb)
            feats_sb = big.tile([P, NT, DP + 1], F16)
            nc.vector.memset(feats_sb[:, :, DP:DP + 1], 1.0)
            # dequant: feats = (u - 128) * s_node, one fused op per tile
            for t in range(NT):
                nc.vector.tensor_scalar(
                    feats_sb[:, t, 0:DP],
                    f8_sb[:, t, :],
                    -128.0,
                    scalar2=fs_sb[:, t:t + 1],
                    op0=ALU.add,
                    op1=ALU.mult,
                )

            # A/B expansion rows rebuilt on device from the minimal 10 rows:
            # A = [hi hi lo -n2hi -n2lo 1 1], B = [2hi 2lo 2hi 1 1 -n2hi -n2lo]
            hi_t = big.tile([4, N], F16)
            nc.sync.dma_start(
                hi_t,
                pkc_view(R_HI, 128).rearrange(
                    "(r jh jl1) jl0 -> r (jh jl1 jl0)", r=4, jh=32, jl1=2
                ),
            )
            lo_t = big.tile([4, N], F16)
            nc.sync.dma_start(
                lo_t,
                pkc_view(R_LO, 128).rearrange(
                    "(r jh jl1) jl0 -> r (jh jl1 jl0)", r=4, jh=32, jl1=2
                ),
            )
            n2n_t = big.tile([2, N], F16)
            nc.sync.dma_start(
                n2n_t,
                pkc_view(R_N2, 64).rearrange(
                    "(r jh jl1) jl0 -> r (jh jl1 jl0)", r=2, jh=32, jl1=2
                ),
            )
            hi2_t = big.tile([4, N], F16)
            nc.scalar.activation(hi2_t, hi_t, AF.Copy, scale=2.0)  # exact in fp16
            lo2_t = big.tile([4, N], F16)
            nc.scalar.activation(lo2_t, lo_t, AF.Copy, scale=2.0)
            ones2 = big.tile([2, N], F16)
            nc.vector.memset(ones2, 1.0)
            A_sb = big.tile([16, N], F16)
            B_sb = big.tile([16, N], F16)
            nc.sync.dma_start(A_sb[0:4, :], hi_t)
            nc.sync.dma_start(A_sb[4:8, :], hi_t)
            nc.sync.dma_start(A_sb[8:12, :], lo_t)
            nc.sync.dma_start(A_sb[12:14, :], n2n_t)
            nc.sync.dma_start(A_sb[14:16, :], ones2)
            nc.sync.dma_start(B_sb[0:4, :], hi2_t)
            nc.sync.dma_start(B_sb[4:8, :], lo2_t)
            nc.sync.dma_start(B_sb[8:12, :], hi2_t)
            nc.sync.dma_start(B_sb[12:14, :], ones2)
            nc.sync.dma_start(B_sb[14:16, :], n2n_t)

            # ---- persistent state ----
            wm_all = big.tile([P, NT * N], F16)    # masked weight row-tiles
            o8_all = big.tile([P, NT * DP], dt.int8)   # quantized wmean tiles
            s_all = big.tile([P, NT], F16)             # per-node wmean scales

            # scratch (fixed addresses; loop back-edge serializes iterations)
            a_t = big.tile([16, P], F16)
            w_t = big.tile([P, N], F32)
            m1 = big.tile([P, 8], F32)
            m2 = big.tile([P, 8], F32)
            w1z = big.tile([P, N], F32)
            sel = big.tile([P, N], F32)
            wmt_t = big.tile([P, N], F16)
            lhs_cols = big.tile([P, NT, P], F16)   # transposed masked weights
            recip = big.tile([P, 1], F32)
            wmf = big.tile([P, DP], F32)
            amax = big.tile([P, 1], F32)
            s32 = big.tile([P, 1], F32)
            rq = big.tile([P, 1], F32)

            s_ps = ps_s.tile([P, N], F32)                  # 4 banks
            agg_ps = ps_agg.tile([P, DP + 1], F32)         # 1 bank

            # ---- Loop1: distances, exp, exact row-side top-16 mask ----
            with tc.For_i(0, NT, 1) as t:
                nc.sync.dma_start(a_t, A_sb[:, ds(t * P, P)])
                for c in range(JC):
                    nc.tensor.matmul(
                        s_ps[:, c * FREE:(c + 1) * FREE],
                        lhsT=a_t,
                        rhs=B_sb[:, c * FREE:(c + 1) * FREE],
                        start=True, stop=True,
                    )
                for c in range(JC):
                    nc.scalar.activation(
                        w_t[:, c * FREE:(c + 1) * FREE],
                        s_ps[:, c * FREE:(c + 1) * FREE],
                        AF.Exp, scale=10.0 / (CSCALE * CSCALE),
                    )
                nc.vector.max(m1, w_t)
                nc.vector.match_replace(
                    w1z, in_to_replace=m1, in_values=w_t, imm_value=0.0,
                )
                nc.vector.max(m2, w1z)
                # exact same-side compare: keeps exactly the top-16 per row
                nc.vector.tensor_scalar(
                    sel, w_t, m2[:, 7:8], scalar2=None, op0=ALU.is_ge
                )
                nc.vector.tensor_mul(wm_all[:, ds(t * N, N)], w_t, sel)

            # ---- Loop2: transpose masked row-tile (exact), aggregate ----
            with tc.For_i(0, NT, 1) as t:
                nc.sync.dma_start(wmt_t, wm_all[:, ds(t * N, N)])
                for jb in range(NT):
                    tp = ps_mlp.tile([P, P], F16, tag="tp")
                    nc.tensor.transpose(
                        tp, wmt_t[:, jb * P:(jb + 1) * P], identh
                    )
                    nc.scalar.activation(lhs_cols[:, jb, :], tp, AF.Copy)
                for jb in range(NT):
                    nc.tensor.matmul(
                        agg_ps,
                        lhsT=lhs_cols[:, jb, :],
                        rhs=feats_sb[:, jb, :],
                        start=(jb == 0), stop=(jb == NT - 1),
                    )
                nc.vector.reciprocal(recip, agg_ps[:, DP:DP + 1])
                nc.vector.tensor_scalar_mul(wmf, agg_ps[:, 0:DP], recip)
                # per-node symmetric int8 quant: s = absmax/126.5 (margin so
                # |q| <= 127 survives the fp16 rounding of s), downloadable
                # scale is the fp16-rounded s, and q uses 1/that so host
                # dequant q*s reproduces wmean exactly up to the int rounding
                nc.vector.reduce_max(
                    amax, wmf, axis=mybir.AxisListType.X,
                    apply_absolute_value=True,
                )
                nc.vector.tensor_scalar_max(amax, amax, 1e-12)
                nc.scalar.activation(
                    s_all[:, ds(t, 1)], amax, AF.Copy, scale=1.0 / 126.5
                )
                nc.scalar.activation(s32, s_all[:, ds(t, 1)], AF.Copy)
                nc.vector.reciprocal(rq, s32)
                nc.vector.tensor_scalar_mul(
                    o8_all[:, ds(t * DP, DP)], wmf, rq
                )

            nc.sync.dma_start(
                out8_d[0:N, :].rearrange("(t p) d -> p t d", p=P),
                o8_all.rearrange("p (t d) -> p t d", t=NT),
            )  # quantized wmean; the 2-layer MLP runs on the host
            nc.sync.dma_start(
                out8_d[N:N + 64, :].rearrange("p2 (a c) -> (p2 a) c", a=2),
                s_all.bitcast(dt.int8),
            )
            if debug:
                nc.sync.dma_start(dbg_w[:, :], wm_all[:, 0:N])
                nc.sync.dma_start(
                    dbg_lhs[:, :],
                    lhs_cols.rearrange("p j i -> p (j i)"),
                )

    return nc


_CACHE = {}


def _get_nc():
    if "nc" not in _CACHE:
        nc = bacc_mod.Bacc()
        build_gravnet(nc)
        nc.finalize()
        _CACHE["nc"] = nc
    return _CACHE["nc"]


class _Exec:
    """Cached SPMD dispatcher: one jitted shard_map over 8 cores.

    Mirrors what run_bass_kernel_spmd -> bass2jax.run_bass_via_pjrt does
    under axon, but builds the jit wrapper (and the device-side zero-buffer
    creator) exactly once, so warm calls pay no re-trace / persistent-cache
    lookup and no H2D upload of donated output buffers.
    """

    def __init__(self, nc):
        bass2jax.install_neuronx_cc_hook()
        self.nc = nc
        partition_name = (
            nc.partition_id_tensor.name if nc.partition_id_tensor else None
        )
        in_names, out_names, out_avals = [], [], []
        self.out_shapes = []
        for alloc in nc.m.functions[0].allocations:
            if not isinstance(alloc, mybir.MemoryLocationSet):
                continue
            name = alloc.memorylocations[0].name
            if alloc.kind == "ExternalInput":
                if name != partition_name:
                    in_names.append(name)
            elif alloc.kind == "ExternalOutput":
                shape = tuple(alloc.tensor_shape)
                np_dt = mybir.dt.np(alloc.dtype)
                out_names.append(name)
                out_avals.append(jax.core.ShapedArray(shape, np_dt))
                self.out_shapes.append((shape, np_dt))
        assert in_names == ["pk"] and out_names == ["out8"], (
            in_names, out_names,
        )
        n_params = len(in_names)
        n_outs = len(out_avals)
        in_names_full = in_names + out_names
        if partition_name is not None:
            in_names_full.append(partition_name)
        donate = tuple(range(n_params, n_params + n_outs))

        def _body(*args):
            operands = list(args)
            if partition_name is not None:
                operands.append(bass2jax.partition_id_tensor())
            outs = bass2jax._bass_exec_p.bind(
                *operands,
                out_avals=tuple(out_avals),
                in_names=tuple(in_names_full),
                out_names=tuple(out_names),
                lowering_input_output_aliases=(),
                sim_require_finite=True,
                sim_require_nnan=True,
                nc=nc,
            )
            return tuple(outs)

        devices = jax.devices()[:B]
        assert len(devices) == B, f"need {B} devices, got {len(jax.devices())}"
        self.mesh = Mesh(np.asarray(devices), ("core",))
        spec = PartitionSpec("core")
        self.sharding = NamedSharding(self.mesh, spec)
        in_specs = (spec,) * (n_params + n_outs)
        out_specs = (spec,) * n_outs
        self.sharded = jax.jit(
            shard_map(
                _body,
                mesh=self.mesh,
                in_specs=in_specs,
                out_specs=out_specs,
                check_rep=False,
            ),
            donate_argnums=donate,
            keep_unused=True,
        )
        # Device-side creation of the donated output buffers (the bass kernel
        # writes every element of both outputs, so contents are irrelevant,
        # but the custom call needs committed buffers to consume).
        gshapes = [((B * s[0],) + s[1:], d) for s, d in self.out_shapes]
        self.zeros = jax.jit(
            lambda: tuple(jnp.zeros(gs, gd) for gs, gd in gshapes),
            out_shardings=(self.sharding,) * n_outs,
        )
        self._next_zeros = None

    def put(self, arr: np.ndarray):
        """Async upload of a (B*rows, ...) host array, sharded over cores."""
        return jax.device_put(arr, self.sharding)

    def run(self, *in_devs):
        """Dispatch and return per-output lists of per-core shards in core
        order, with async D2H copies already issued (single round trip)."""
        # donated buffers are input-independent: use the set pre-dispatched
        # at the end of the previous call when available
        zs = self._next_zeros if self._next_zeros is not None else self.zeros()
        outs = self.sharded(*in_devs, *zs)
        # pre-dispatch (async, device-side) the next call's donated buffers
        self._next_zeros = self.zeros()
        all_shards = []
        for og in outs:
            shards = sorted(
                og.addressable_shards, key=lambda s: s.index[0].start or 0
            )
            all_shards.append([s.data for s in shards])
        for datas in all_shards:
            for d in datas:
                d.copy_to_host_async()
        return all_shards


def _get_exec():
    if "exec" not in _CACHE:
        _CACHE["exec"] = _Exec(_get_nc())
    return _CACHE["exec"]


def _pack_coords(coords, s16):
    """Packed fp16 coords input [B*RC_END, 64] (view of a cached buffer).

    d2 expansion uses an fp16 hi/lo split of coords (and |c|^2) so the PE
    contraction (exact fp16 products, f32 accumulate) reproduces f32-accurate
    s = -d2:  s = sum_r 2(hi+lo)_i (hi+lo)_j - n2_i - n2_j, dropping lo*lo.
    """
    n2 = np.sum(coords * coords, axis=-1)                # [B,N]
    c_hi = coords.astype(np.float16).astype(np.float32)
    c_lo = coords - c_hi
    n2_hi = n2.astype(np.float16).astype(np.float32)
    n2_lo = n2 - n2_hi
    cT_hi = c_hi.transpose(0, 2, 1)                      # [B,4,N]
    cT_lo = c_lo.transpose(0, 2, 1)

    buf = _CACHE.get("pkc_all")
    if buf is None:
        buf = np.empty((B, RC_END, 64), np.float16)
        _CACHE["pkc_all"] = buf
    pkc = buf
    pkc[:, R_HI:R_HI + 128] = cT_hi.reshape(B, 128, 64)
    pkc[:, R_LO:R_LO + 128] = cT_lo.reshape(B, 128, 64)
    n2n = pkc[:, R_N2:R_N2 + 64].reshape(B, 2, N)        # view
    n2n[:, 0] = -n2_hi
    n2n[:, 1] = -n2_lo
    pkc[:, R_FS:R_FS + 32] = (
        s16.reshape(B, NT, P).transpose(0, 2, 1).reshape(B, 32, 64)
    )
    return pkc.reshape(B * RC_END, 64)


def kernel(**inputs) -> np.ndarray:
    x = np.asarray(inputs["x"], dtype=np.float32)
    mask = np.asarray(inputs["mask"])
    W_space = np.asarray(inputs["W_space"], np.float32)
    b_space = np.asarray(inputs["b_space"], np.float32)
    W_feat = np.asarray(inputs["W_feat"], np.float32)
    b_feat = np.asarray(inputs["b_feat"], np.float32)
    W1 = np.asarray(inputs["W1"], np.float32)
    W2 = np.asarray(inputs["W2"], np.float32)
    b1 = np.asarray(inputs["b1"], np.float32)
    b2 = np.asarray(inputs["b2"], np.float32)

    ex = _get_exec()
    xf = x.reshape(B * N, DIN)

    # feats first: its uint8 copy is most of the upload bytes, so ship it the
    # moment it exists (device_put is async -- serialization+network stream
    # while the host packs coords below). Per-node symmetric quantization:
    # u = round(f/s) + 128 with s = absmax/126.5 rounded to fp16 (the device
    # dequantizes with that same fp16 s, so |u-128| <= 127 is guaranteed).
    # The +128.5 shift makes uint8 truncation equal round-to-nearest.
    bufs = _CACHE.get("host_bufs")
    if bufs is None:
        bufs = {
            "fq": np.empty((B * N, DP), np.float32),
            "pk": np.empty((B, N + 2 * RC_END, DP), np.uint8),
            "h": np.empty((B * N, DOUT), np.float32),
            "wm32": np.empty((N, DP), np.float32),
        }
        _CACHE["host_bufs"] = bufs
    fq, pk = bufs["fq"], bufs["pk"]

    feats = xf @ W_feat                                  # [B*N, DP] f32
    if b_feat.any():
        feats += b_feat
    if not mask.all():
        feats *= mask.reshape(B * N, 1)
    s = feats.max(axis=1)
    smin = feats.min(axis=1)
    np.negative(smin, out=smin)
    np.maximum(s, smin, out=s)
    np.maximum(s, 1e-12, out=s)
    s /= 126.5
    s16 = s.astype(np.float16)                           # [B*N]
    s32 = s16.astype(np.float32)
    np.multiply(feats, (1.0 / s32)[:, None], out=fq)
    fq += 128.5
    # trunc of (v+128.5) == round(v)+128; NB pk[:, 0:N] is a strided view,
    # copyto writes through it (a .reshape here would silently copy)
    np.copyto(pk[:, 0:N], fq.reshape(B, N, DP), casting="unsafe")

    coords = (xf @ (W_space * CSCALE)).reshape(B, N, DS)
    if b_space.any():
        coords += b_space * CSCALE
    pkc = _pack_coords(coords, s16).reshape(B, RC_END * 64)
    pk[:, N:].reshape(B, -1)[:] = pkc.view(np.uint8)

    pk_dev = ex.put(pk.reshape(B * (N + 2 * RC_END), DP))
    (q_shards,) = ex.run(pk_dev)

    # MLP: out = relu([feats|wmean] @ W1 + b1) @ W2 + b2. The feats half
    # doesn't need device results -- compute it while the tunnel round trip
    # is in flight, then pipeline the per-batch tail against the per-shard
    # download stream (network-bound, so the CPU is free).
    W1a, W1b = W1[:DP], W1[DP:]
    h = bufs["h"]
    np.matmul(feats, W1a, out=h)
    h += b1
    h = h.reshape(B, N, DOUT)
    # fresh output each call -- the caller may hold results across calls
    out = np.empty((B, N, DOUT), np.float32)
    wm32 = bufs["wm32"]
    for b in range(B):
        sh = np.asarray(q_shards[b])                     # [N+64, DP] int8
        np.copyto(wm32, sh[0:N], casting="unsafe")
        ws = sh[N:].reshape(-1).view(np.float16).reshape(P, NT)
        wm32 *= ws.astype(np.float32).T.reshape(N, 1)    # node n = t*P + p
        hb = h[b]
        hb += wm32 @ W1b
        np.maximum(hb, 0.0, out=hb)
        np.matmul(hb, W2, out=out[b])
        out[b] += b2
    return out


if __name__ == "__main__":
    rng = np.random.default_rng(0)
    ins = {
        "x": rng.standard_normal((8, N, DIN), dtype=np.float32),
        "mask": np.ones((8, N), bool),
        "W_space": rng.standard_normal((DIN, DS), dtype=np.float32) * 0.02,
        "b_space": np.zeros(DS, np.float32),
        "W_feat": rng.standard_normal((DIN, DP), dtype=np.float32) * 0.02,
        "b_feat": np.zeros(DP, np.float32),
        "W1": rng.standard_normal((2 * DP, DOUT), dtype=np.float32) * 0.02,
        "b1": np.zeros(DOUT, np.float32),
        "W2": rng.standard_normal((DOUT, DOUT), dtype=np.float32) * 0.02,
        "b2": np.zeros(DOUT, np.float32),
    }
    print(kernel(**ins).shape)
